# revision 6
# baseline (speedup 1.0000x reference)
"""RPN (FPN region proposal network) kernel for Trainium2, 8 NeuronCores.

Device: full conv tower (3x3 conv + ReLU + cls/bbox heads) for all 5 FPN
levels, data-parallel over rows across 8 cores (fp32 PE matmuls).
Host: shard/unshard + the sequential top-k/NMS/merge bookkeeping in fp32.
"""

import math
import numpy as np

import concourse.bacc as bacc
import concourse.tile as tile
import concourse.mybir as mybir
from concourse.bass_utils import run_bass_kernel_spmd

# ---- hardcoded problem config ----
B = 2
C = 256
HW = [(160, 256), (80, 128), (40, 64), (20, 32), (10, 16)]
SIZES = [32, 64, 128, 256, 512]
STRIDES = [4, 8, 16, 32, 64]
RATIOS = np.array([0.2323283, 0.63365731, 1.28478321, 3.15089189], dtype=np.float64)
A = 4
PRE_NMS_TOP_N = 6000
POST_NMS_TOP_N = 1000
FPN_POST_NMS_TOP_N = 1000
NMS_THRESH = 0.7
BBOX_XFORM_CLIP = math.log(1000.0 / 16.0)
NEG = -1e9
NCORES = 8
# rows per core per image for each level (uniform across cores; padded)
NROWS = [20, 10, 5, 3, 2]
DT = mybir.dt.float32
import os
USE_F32R = os.environ.get("RPN_F32R", "0") == "1"

_CACHE = {}


def _mmdt(ap):
    return ap


def _build_program():
    nc = bacc.Bacc("TRN2", target_bir_lowering=False, debug=False,
                   num_devices=NCORES)
    dt = DT
    mdt = mybir.dt.float32r if USE_F32R else DT
    f_in = []
    o_out = []
    for l, (H, W) in enumerate(HW):
        n = NROWS[l]
        f_in.append(nc.dram_tensor(f"f{l}", [B, C, n + 2, W + 2], mdt,
                                   kind="ExternalInput").ap())
        o_out.append(nc.dram_tensor(f"o{l}", [B, 20, n * W], dt,
                                    kind="ExternalOutput").ap())
    wconv = nc.dram_tensor("wconv", [128, 36, 128], mdt, kind="ExternalInput").ap()
    whead = nc.dram_tensor("whead", [128, 2, 20], mdt, kind="ExternalInput").ap()
    bconv = nc.dram_tensor("bconv", [128, 2], dt, kind="ExternalInput").ap()
    bhead = nc.dram_tensor("bhead", [20, 1], dt, kind="ExternalInput").ap()

    with tile.TileContext(nc) as tc:
        with (
            tc.tile_pool(name="wpool", bufs=1) as wpool,
            tc.tile_pool(name="feat", bufs=2) as fpool,
            tc.tile_pool(name="tbuf", bufs=3) as tpool,
            tc.tile_pool(name="outb", bufs=3) as opool,
            tc.tile_pool(name="psum", bufs=2, space="PSUM") as ppool,
            tc.tile_pool(name="psumh", bufs=2, space="PSUM") as phpool,
        ):
            wsb = wpool.tile([128, 36 * 128], mdt, tag="wconv")
            nc.sync.dma_start(out=wsb[:], in_=wconv[:, :, :])
            whsb = wpool.tile([128, 2 * 20], mdt, tag="whead")
            nc.sync.dma_start(out=whsb[:], in_=whead[:, :, :])
            bcsb = wpool.tile([128, 2], dt, tag="bconv")
            nc.sync.dma_start(out=bcsb[:], in_=bconv[:, :])
            bhsb = wpool.tile([20, 1], dt, tag="bhead")
            nc.sync.dma_start(out=bhsb[:], in_=bhead[:, :])

            def widx(ky, kx, ci, co):
                i = ((ky * 3 + kx) * 2 + ci) * 2 + co
                return wsb[:, i * 128:(i + 1) * 128]

            for l, (H, W) in enumerate(HW):
                n = NROWS[l]
                Wp = W + 2
                rpt = max(1, min(n, 512 // W))  # rows per tile
                for img in range(B):
                    fh = []
                    for ci in range(2):
                        ft = fpool.tile([128, (n + 2) * Wp], mdt, tag=f"feat{l}")
                        nc.sync.dma_start(
                            out=ft[:],
                            in_=f_in[l][img, ci * 128:(ci + 1) * 128, :, :])
                        fh.append(ft)
                    r0 = 0
                    while r0 < n:
                        rt = min(rpt, n - r0)
                        N = rt * W
                        tsb = []
                        for co in range(2):
                            ps = ppool.tile([128, N], dt, tag="convps")
                            first = True
                            for ci in range(2):
                                fv = fh[ci][:].rearrange(
                                    "p (r w) -> p r w", w=Wp)
                                for ky in range(3):
                                    for kx in range(3):
                                        rhs = fv[:, r0 + ky:r0 + ky + rt,
                                                 kx:kx + W]
                                        nc.tensor.matmul(
                                            ps[:], _mmdt(widx(ky, kx, ci, co)),
                                            _mmdt(rhs),
                                            start=first,
                                            stop=(ci == 1 and ky == 2 and kx == 2))
                                        first = False
                            ts = tpool.tile([128, N], mdt, tag="tbuf")
                            nc.scalar.activation(
                                ts[:], ps[:],
                                mybir.ActivationFunctionType.Relu,
                                bias=bcsb[:, co:co + 1], scale=1.0)
                            tsb.append(ts)
                        ph = phpool.tile([20, N], dt, tag="headps")
                        for h in range(2):
                            nc.tensor.matmul(
                                ph[:], _mmdt(whsb[:, h * 20:(h + 1) * 20]),
                                _mmdt(tsb[h][:]),
                                start=(h == 0), stop=(h == 1))
                        ob = opool.tile([20, N], dt, tag="outb")
                        nc.scalar.activation(
                            ob[:], ph[:],
                            mybir.ActivationFunctionType.Identity,
                            bias=bhsb[:, 0:1], scale=1.0)
                        nc.sync.dma_start(
                            out=o_out[l][img, :, r0 * W:(r0 + rt) * W],
                            in_=ob[:])
                        r0 += rt
    nc.compile()
    return nc


def _prep_shards(feats, conv_w, conv_b, cls_w, bbox_w, cls_b, bbox_b):
    # weights
    wconv = np.zeros((128, 36, 128), np.float32)
    for ky in range(3):
        for kx in range(3):
            for ci in range(2):
                for co in range(2):
                    i = ((ky * 3 + kx) * 2 + ci) * 2 + co
                    wconv[:, i, :] = conv_w[co * 128:(co + 1) * 128,
                                            ci * 128:(ci + 1) * 128,
                                            ky, kx].T
    Wh = np.concatenate([cls_w[:, :, 0, 0], bbox_w[:, :, 0, 0]], axis=0)  # [20,256]
    whead = np.zeros((128, 2, 20), np.float32)
    for ci in range(2):
        whead[:, ci, :] = Wh[:, ci * 128:(ci + 1) * 128].T
    bconv = np.stack([conv_b[:128], conv_b[128:]], axis=1).astype(np.float32)
    bhead = np.concatenate([cls_b, bbox_b]).astype(np.float32)[:, None]

    in_maps = []
    for c in range(NCORES):
        m = {"wconv": wconv, "whead": whead, "bconv": bconv, "bhead": bhead}
        for l, (H, W) in enumerate(HW):
            n = NROWS[l]
            fpad = np.zeros((B, C, n + 2, W + 2), np.float32)
            start = c * n  # feature-row index of first owned row
            # rows [start-1, start+n+1) of the feature map, zero-padded
            lo = max(start - 1, 0)
            hi = min(start + n + 1, H)
            if hi > lo:
                fpad[:, :, lo - (start - 1):hi - (start - 1),
                     1:W + 1] = feats[l][:, :, lo:hi, :]
            m[f"f{l}"] = fpad
        in_maps.append(m)
    return in_maps


def _run_device(feats, conv_w, conv_b, cls_w, bbox_w, cls_b, bbox_b):
    if "nc" not in _CACHE:
        _CACHE["nc"] = _build_program()
    nc = _CACHE["nc"]
    in_maps = _prep_shards(feats, conv_w, conv_b, cls_w, bbox_w, cls_b, bbox_b)
    trace = os.environ.get("RPN_TRACE", "0") == "1"
    res = run_bass_kernel_spmd(nc, in_maps, core_ids=list(range(NCORES)),
                               trace=trace)
    if trace:
        print("exec_time_ns:", res.exec_time_ns)
        _CACHE["last"] = res
    # reassemble per level
    objs, regs = [], []
    for l, (H, W) in enumerate(HW):
        n = NROWS[l]
        full = np.zeros((B, 20, H, W), np.float32)
        for c in range(NCORES):
            start = c * n
            take = min(n, max(0, H - start))
            if take <= 0:
                continue
            chunk = res.results[c][f"o{l}"].reshape(B, 20, n, W)
            full[:, :, start:start + take, :] = chunk[:, :, :take, :]
        objs.append(full[:, :4])     # [B,4,H,W]
        regs.append(full[:, 4:20])   # [B,16,H,W]
    return objs, regs


# ---- host post-processing (fp32, mirrors the reference semantics) ----

def _level_anchors(size, stride, H, W):
    area = float(size) ** 2
    ws = np.sqrt(area / RATIOS)
    hs = ws * RATIOS
    base = np.stack([-ws / 2, -hs / 2, ws / 2, hs / 2], axis=1)
    sx = np.arange(W, dtype=np.float64) * stride
    sy = np.arange(H, dtype=np.float64) * stride
    gx, gy = np.meshgrid(sx, sy)
    shifts = np.stack([gx, gy, gx, gy], axis=-1)
    anch = shifts[:, :, None, :] + base[None, None, :, :]
    return anch.reshape(-1, 4).astype(np.float32)


def _decode_clip(rel, anchors, img_h, img_w):
    f32 = np.float32
    wa = anchors[:, 2] - anchors[:, 0] + f32(1.0)
    ha = anchors[:, 3] - anchors[:, 1] + f32(1.0)
    cxa = anchors[:, 0] + f32(0.5) * wa
    cya = anchors[:, 1] + f32(0.5) * ha
    dx, dy = rel[:, 0], rel[:, 1]
    dw = np.minimum(rel[:, 2], f32(BBOX_XFORM_CLIP))
    dh = np.minimum(rel[:, 3], f32(BBOX_XFORM_CLIP))
    cx = dx * wa + cxa
    cy = dy * ha + cya
    w = np.exp(dw) * wa
    h = np.exp(dh) * ha
    boxes = np.stack([cx - f32(0.5) * w, cy - f32(0.5) * h,
                      cx + f32(0.5) * w - f32(1.0),
                      cy + f32(0.5) * h - f32(1.0)], axis=1)
    x1 = np.clip(boxes[:, 0], f32(0.0), f32(img_w - 1.0))
    y1 = np.clip(boxes[:, 1], f32(0.0), f32(img_h - 1.0))
    x2 = np.clip(boxes[:, 2], f32(0.0), f32(img_w - 1.0))
    y2 = np.clip(boxes[:, 3], f32(0.0), f32(img_h - 1.0))
    return np.stack([x1, y1, x2, y2], axis=1)


def _nms_keep(boxes, scores, n_out, thresh):
    f32 = np.float32
    area = (boxes[:, 2] - boxes[:, 0]) * (boxes[:, 3] - boxes[:, 1])
    s = scores.copy()
    keep = np.empty(n_out, np.int64)
    for it in range(n_out):
        i = int(np.argmax(s))
        valid = s[i] > f32(NEG * 0.5)
        b = boxes[i]
        xx1 = np.maximum(b[0], boxes[:, 0])
        yy1 = np.maximum(b[1], boxes[:, 1])
        xx2 = np.minimum(b[2], boxes[:, 2])
        yy2 = np.minimum(b[3], boxes[:, 3])
        inter = np.maximum(xx2 - xx1, f32(0.0)) * np.maximum(yy2 - yy1, f32(0.0))
        iou = inter / (area[i] + area - inter + f32(1e-9))
        s[iou > f32(thresh)] = f32(NEG)
        s[i] = f32(NEG)
        keep[it] = i if valid else -1
    return keep


def _process_level_image(obj_l, reg_l, anchors, img_h, img_w):
    f32 = np.float32
    N = obj_l.shape[0]
    scores = (f32(1.0) / (f32(1.0) + np.exp(-obj_l))).astype(np.float32)
    k = min(PRE_NMS_TOP_N, N)
    order = np.argsort(-scores, kind="stable")
    topi = order[:k]
    topv = scores[topi]
    boxes = _decode_clip(reg_l[topi], anchors[topi], img_h, img_w)
    n_out = min(POST_NMS_TOP_N, k)
    keep = _nms_keep(boxes, topv, n_out, NMS_THRESH)
    safe = np.maximum(keep, 0)
    sel_b = boxes[safe]
    sel_s = np.where(keep >= 0, topv[safe], f32(NEG)).astype(np.float32)
    if n_out < POST_NMS_TOP_N:
        pad = POST_NMS_TOP_N - n_out
        sel_b = np.concatenate([sel_b, np.zeros((pad, 4), np.float32)], axis=0)
        sel_s = np.concatenate([sel_s, np.full((pad,), f32(NEG), np.float32)])
    return sel_b, sel_s


def kernel(feat0, feat1, feat2, feat3, feat4, conv_w, conv_b, cls_w, cls_b,
           bbox_w, bbox_b, img_h, img_w):
    feats = [np.asarray(f, np.float32) for f in
             (feat0, feat1, feat2, feat3, feat4)]
    img_h = int(np.asarray(img_h)); img_w = int(np.asarray(img_w))
    objs, regs = _run_device(feats, np.asarray(conv_w, np.float32),
                             np.asarray(conv_b, np.float32),
                             np.asarray(cls_w, np.float32),
                             np.asarray(bbox_w, np.float32),
                             np.asarray(cls_b, np.float32),
                             np.asarray(bbox_b, np.float32))
    all_b, all_s = [], []
    for l, (H, W) in enumerate(HW):
        obj = np.transpose(objs[l], (0, 2, 3, 1)).reshape(B, H * W * A)
        reg = np.transpose(regs[l].reshape(B, A, 4, H, W),
                           (0, 3, 4, 1, 2)).reshape(B, H * W * A, 4)
        anchors = _level_anchors(SIZES[l], STRIDES[l], H, W)
        bs, ss = [], []
        for b in range(B):
            sb, sv = _process_level_image(obj[b], reg[b], anchors, img_h, img_w)
            bs.append(sb); ss.append(sv)
        all_b.append(np.stack(bs)); all_s.append(np.stack(ss))
    boxes = np.concatenate(all_b, axis=1)   # [B, 5000, 4]
    scores = np.concatenate(all_s, axis=1)  # [B, 5000]
    out = np.empty((B, FPN_POST_NMS_TOP_N, 5), np.float32)
    for b in range(B):
        order = np.argsort(-scores[b], kind="stable")[:FPN_POST_NMS_TOP_N]
        out[b, :, :4] = boxes[b][order]
        out[b, :, 4] = scores[b][order]
    return out


# revision 9
# speedup vs baseline: 1.0019x; 1.0019x over previous
"""RPN (FPN region proposal network) kernel for Trainium2, 8 NeuronCores.

Device: full conv tower (3x3 conv + ReLU + cls/bbox heads) for all 5 FPN
levels, data-parallel over rows across 8 cores (fp32 PE matmuls).
Host: shard/unshard + the sequential top-k/NMS/merge bookkeeping in fp32.
"""

import math
import numpy as np

import concourse.bacc as bacc
import concourse.tile as tile
import concourse.mybir as mybir
from concourse.bass_utils import run_bass_kernel_spmd

# ---- hardcoded problem config ----
B = 2
C = 256
HW = [(160, 256), (80, 128), (40, 64), (20, 32), (10, 16)]
SIZES = [32, 64, 128, 256, 512]
STRIDES = [4, 8, 16, 32, 64]
RATIOS = np.array([0.2323283, 0.63365731, 1.28478321, 3.15089189], dtype=np.float64)
A = 4
PRE_NMS_TOP_N = 6000
POST_NMS_TOP_N = 1000
FPN_POST_NMS_TOP_N = 1000
NMS_THRESH = 0.7
BBOX_XFORM_CLIP = math.log(1000.0 / 16.0)
NEG = -1e9
NCORES = 8
# rows per core per image for each level (uniform across cores; padded)
NROWS = [20, 10, 5, 3, 2]
DT = mybir.dt.float32
import os
USE_F32R = os.environ.get("RPN_F32R", "0") == "1"

_CACHE = {}


def _mmdt(ap):
    return ap


def _build_program():
    nc = bacc.Bacc("TRN2", target_bir_lowering=False, debug=False,
                   num_devices=NCORES)
    dt = DT
    mdt = mybir.dt.float32r if USE_F32R else DT
    f_in = []
    o_out = []
    for l, (H, W) in enumerate(HW):
        n = NROWS[l]
        f_in.append(nc.dram_tensor(f"f{l}", [B, C, n + 2, W + 2], mdt,
                                   kind="ExternalInput").ap())
        o_out.append(nc.dram_tensor(f"o{l}", [B, 20, n * W], dt,
                                    kind="ExternalOutput").ap())
    wconv = nc.dram_tensor("wconv", [128, 36, 128], mdt, kind="ExternalInput").ap()
    whead = nc.dram_tensor("whead", [128, 2, 20], mdt, kind="ExternalInput").ap()
    bconv = nc.dram_tensor("bconv", [128, 2], dt, kind="ExternalInput").ap()
    bhead = nc.dram_tensor("bhead", [20, 1], dt, kind="ExternalInput").ap()

    with tile.TileContext(nc) as tc:
        with (
            tc.tile_pool(name="wpool", bufs=1) as wpool,
            tc.tile_pool(name="feat", bufs=2) as fpool,
            tc.tile_pool(name="tbuf", bufs=3) as tpool,
            tc.tile_pool(name="outb", bufs=3) as opool,
            tc.tile_pool(name="psum", bufs=3, space="PSUM") as ppool,
            tc.tile_pool(name="psumh", bufs=2, space="PSUM") as phpool,
        ):
            wsb = wpool.tile([128, 36 * 128], mdt, tag="wconv")
            nc.sync.dma_start(out=wsb[:], in_=wconv[:, :, :])
            whsb = wpool.tile([128, 2 * 20], mdt, tag="whead")
            nc.sync.dma_start(out=whsb[:], in_=whead[:, :, :])
            bcsb = wpool.tile([128, 2], dt, tag="bconv")
            nc.sync.dma_start(out=bcsb[:], in_=bconv[:, :])
            bhsb = wpool.tile([20, 1], dt, tag="bhead")
            nc.sync.dma_start(out=bhsb[:], in_=bhead[:, :])

            def widx(ky, kx, ci, co):
                i = ((ky * 3 + kx) * 2 + ci) * 2 + co
                return wsb[:, i * 128:(i + 1) * 128]

            for l in (4, 3, 2, 1, 0):
                H, W = HW[l]
                n = NROWS[l]
                Wp = W + 2
                rpt = max(1, min(n, 512 // W))  # rows per tile
                for img in range(B):
                    fh = []
                    for ci in range(2):
                        ft = fpool.tile([128, (n + 2) * Wp], mdt, tag=f"feat{l}")
                        nc.sync.dma_start(
                            out=ft[:],
                            in_=f_in[l][img, ci * 128:(ci + 1) * 128, :, :])
                        fh.append(ft)
                    r0 = 0
                    while r0 < n:
                        rt = min(rpt, n - r0)
                        N = rt * W
                        tsb = []
                        for co in range(2):
                            ps = ppool.tile([128, N], dt, tag="convps")
                            first = True
                            for ci in range(2):
                                fv = fh[ci][:].rearrange(
                                    "p (r w) -> p r w", w=Wp)
                                for ky in range(3):
                                    for kx in range(3):
                                        rhs = fv[:, r0 + ky:r0 + ky + rt,
                                                 kx:kx + W]
                                        nc.tensor.matmul(
                                            ps[:], _mmdt(widx(ky, kx, ci, co)),
                                            _mmdt(rhs),
                                            start=first,
                                            stop=(ci == 1 and ky == 2 and kx == 2))
                                        first = False
                            ts = tpool.tile([128, N], mdt, tag="tbuf")
                            nc.scalar.activation(
                                ts[:], ps[:],
                                mybir.ActivationFunctionType.Relu,
                                bias=bcsb[:, co:co + 1], scale=1.0)
                            tsb.append(ts)
                        ph = phpool.tile([20, N], dt, tag="headps")
                        for h in range(2):
                            nc.tensor.matmul(
                                ph[:], _mmdt(whsb[:, h * 20:(h + 1) * 20]),
                                _mmdt(tsb[h][:]),
                                start=(h == 0), stop=(h == 1))
                        ob = opool.tile([20, N], dt, tag="outb")
                        nc.scalar.activation(
                            ob[:], ph[:],
                            mybir.ActivationFunctionType.Identity,
                            bias=bhsb[:, 0:1], scale=1.0)
                        nc.sync.dma_start(
                            out=o_out[l][img, :, r0 * W:(r0 + rt) * W],
                            in_=ob[:])
                        r0 += rt
    nc.compile()
    return nc


def _prep_shards(feats, conv_w, conv_b, cls_w, bbox_w, cls_b, bbox_b):
    # weights
    wconv = np.zeros((128, 36, 128), np.float32)
    for ky in range(3):
        for kx in range(3):
            for ci in range(2):
                for co in range(2):
                    i = ((ky * 3 + kx) * 2 + ci) * 2 + co
                    wconv[:, i, :] = conv_w[co * 128:(co + 1) * 128,
                                            ci * 128:(ci + 1) * 128,
                                            ky, kx].T
    Wh = np.concatenate([cls_w[:, :, 0, 0], bbox_w[:, :, 0, 0]], axis=0)  # [20,256]
    whead = np.zeros((128, 2, 20), np.float32)
    for ci in range(2):
        whead[:, ci, :] = Wh[:, ci * 128:(ci + 1) * 128].T
    bconv = np.stack([conv_b[:128], conv_b[128:]], axis=1).astype(np.float32)
    bhead = np.concatenate([cls_b, bbox_b]).astype(np.float32)[:, None]

    in_maps = []
    for c in range(NCORES):
        m = {"wconv": wconv, "whead": whead, "bconv": bconv, "bhead": bhead}
        for l, (H, W) in enumerate(HW):
            n = NROWS[l]
            fpad = np.zeros((B, C, n + 2, W + 2), np.float32)
            start = c * n  # feature-row index of first owned row
            # rows [start-1, start+n+1) of the feature map, zero-padded
            lo = max(start - 1, 0)
            hi = min(start + n + 1, H)
            if hi > lo:
                fpad[:, :, lo - (start - 1):hi - (start - 1),
                     1:W + 1] = feats[l][:, :, lo:hi, :]
            m[f"f{l}"] = fpad
        in_maps.append(m)
    return in_maps


def _run_device(feats, conv_w, conv_b, cls_w, bbox_w, cls_b, bbox_b):
    if "nc" not in _CACHE:
        _CACHE["nc"] = _build_program()
    nc = _CACHE["nc"]
    in_maps = _prep_shards(feats, conv_w, conv_b, cls_w, bbox_w, cls_b, bbox_b)
    trace = os.environ.get("RPN_TRACE", "0") == "1"
    res = run_bass_kernel_spmd(nc, in_maps, core_ids=list(range(NCORES)),
                               trace=trace)
    if trace:
        print("exec_time_ns:", res.exec_time_ns)
        _CACHE["last"] = res
    # reassemble per level
    objs, regs = [], []
    for l, (H, W) in enumerate(HW):
        n = NROWS[l]
        full = np.zeros((B, 20, H, W), np.float32)
        for c in range(NCORES):
            start = c * n
            take = min(n, max(0, H - start))
            if take <= 0:
                continue
            chunk = res.results[c][f"o{l}"].reshape(B, 20, n, W)
            full[:, :, start:start + take, :] = chunk[:, :, :take, :]
        objs.append(full[:, :4])     # [B,4,H,W]
        regs.append(full[:, 4:20])   # [B,16,H,W]
    return objs, regs


# ---- host post-processing (fp32, mirrors the reference semantics) ----

def _level_anchors(size, stride, H, W):
    area = float(size) ** 2
    ws = np.sqrt(area / RATIOS)
    hs = ws * RATIOS
    base = np.stack([-ws / 2, -hs / 2, ws / 2, hs / 2], axis=1)
    sx = np.arange(W, dtype=np.float64) * stride
    sy = np.arange(H, dtype=np.float64) * stride
    gx, gy = np.meshgrid(sx, sy)
    shifts = np.stack([gx, gy, gx, gy], axis=-1)
    anch = shifts[:, :, None, :] + base[None, None, :, :]
    return anch.reshape(-1, 4).astype(np.float32)


def _decode_clip(rel, anchors, img_h, img_w):
    f32 = np.float32
    wa = anchors[:, 2] - anchors[:, 0] + f32(1.0)
    ha = anchors[:, 3] - anchors[:, 1] + f32(1.0)
    cxa = anchors[:, 0] + f32(0.5) * wa
    cya = anchors[:, 1] + f32(0.5) * ha
    dx, dy = rel[:, 0], rel[:, 1]
    dw = np.minimum(rel[:, 2], f32(BBOX_XFORM_CLIP))
    dh = np.minimum(rel[:, 3], f32(BBOX_XFORM_CLIP))
    cx = dx * wa + cxa
    cy = dy * ha + cya
    w = np.exp(dw) * wa
    h = np.exp(dh) * ha
    boxes = np.stack([cx - f32(0.5) * w, cy - f32(0.5) * h,
                      cx + f32(0.5) * w - f32(1.0),
                      cy + f32(0.5) * h - f32(1.0)], axis=1)
    x1 = np.clip(boxes[:, 0], f32(0.0), f32(img_w - 1.0))
    y1 = np.clip(boxes[:, 1], f32(0.0), f32(img_h - 1.0))
    x2 = np.clip(boxes[:, 2], f32(0.0), f32(img_w - 1.0))
    y2 = np.clip(boxes[:, 3], f32(0.0), f32(img_h - 1.0))
    return np.stack([x1, y1, x2, y2], axis=1)


def _nms_keep(boxes, scores, n_out, thresh):
    f32 = np.float32
    area = (boxes[:, 2] - boxes[:, 0]) * (boxes[:, 3] - boxes[:, 1])
    s = scores.copy()
    keep = np.empty(n_out, np.int64)
    for it in range(n_out):
        i = int(np.argmax(s))
        valid = s[i] > f32(NEG * 0.5)
        b = boxes[i]
        xx1 = np.maximum(b[0], boxes[:, 0])
        yy1 = np.maximum(b[1], boxes[:, 1])
        xx2 = np.minimum(b[2], boxes[:, 2])
        yy2 = np.minimum(b[3], boxes[:, 3])
        inter = np.maximum(xx2 - xx1, f32(0.0)) * np.maximum(yy2 - yy1, f32(0.0))
        iou = inter / (area[i] + area - inter + f32(1e-9))
        s[iou > f32(thresh)] = f32(NEG)
        s[i] = f32(NEG)
        keep[it] = i if valid else -1
    return keep


def _process_level_image(obj_l, reg_l, anchors, img_h, img_w):
    f32 = np.float32
    N = obj_l.shape[0]
    scores = (f32(1.0) / (f32(1.0) + np.exp(-obj_l))).astype(np.float32)
    k = min(PRE_NMS_TOP_N, N)
    order = np.argsort(-scores, kind="stable")
    topi = order[:k]
    topv = scores[topi]
    boxes = _decode_clip(reg_l[topi], anchors[topi], img_h, img_w)
    n_out = min(POST_NMS_TOP_N, k)
    keep = _nms_keep(boxes, topv, n_out, NMS_THRESH)
    safe = np.maximum(keep, 0)
    sel_b = boxes[safe]
    sel_s = np.where(keep >= 0, topv[safe], f32(NEG)).astype(np.float32)
    if n_out < POST_NMS_TOP_N:
        pad = POST_NMS_TOP_N - n_out
        sel_b = np.concatenate([sel_b, np.zeros((pad, 4), np.float32)], axis=0)
        sel_s = np.concatenate([sel_s, np.full((pad,), f32(NEG), np.float32)])
    return sel_b, sel_s


def _post_jax(objs, regs, img_h, img_w):
    """Exact replica of the reference post-conv pipeline on jax-CPU."""
    import jax
    import jax.numpy as jnp
    cpu = jax.devices("cpu")[0]

    def level_anchors(size, stride, H, W):
        area = float(size) ** 2
        ws = np.sqrt(area / RATIOS)
        hs = ws * RATIOS
        base = np.stack([-ws / 2, -hs / 2, ws / 2, hs / 2], axis=1)
        sx = np.arange(W, dtype=np.float64) * stride
        sy = np.arange(H, dtype=np.float64) * stride
        gx, gy = np.meshgrid(sx, sy)
        shifts = np.stack([gx, gy, gx, gy], axis=-1)
        anch = shifts[:, :, None, :] + base[None, None, :, :]
        return jnp.asarray(anch.reshape(-1, 4), dtype=jnp.float32)

    def decode_boxes(rel, anchors):
        wa = anchors[:, 2] - anchors[:, 0] + 1.0
        ha = anchors[:, 3] - anchors[:, 1] + 1.0
        cxa = anchors[:, 0] + 0.5 * wa
        cya = anchors[:, 1] + 0.5 * ha
        dx, dy = rel[:, 0], rel[:, 1]
        dw = jnp.minimum(rel[:, 2], BBOX_XFORM_CLIP)
        dh = jnp.minimum(rel[:, 3], BBOX_XFORM_CLIP)
        cx = dx * wa + cxa
        cy = dy * ha + cya
        w = jnp.exp(dw) * wa
        h = jnp.exp(dh) * ha
        return jnp.stack([cx - 0.5 * w, cy - 0.5 * h,
                          cx + 0.5 * w - 1.0, cy + 0.5 * h - 1.0], axis=1)

    def clip_boxes(boxes):
        x1 = jnp.clip(boxes[:, 0], 0.0, img_w - 1.0)
        y1 = jnp.clip(boxes[:, 1], 0.0, img_h - 1.0)
        x2 = jnp.clip(boxes[:, 2], 0.0, img_w - 1.0)
        y2 = jnp.clip(boxes[:, 3], 0.0, img_h - 1.0)
        return jnp.stack([x1, y1, x2, y2], axis=1)

    def nms_keep(boxes, scores, n_out, thresh):
        area = (boxes[:, 2] - boxes[:, 0]) * (boxes[:, 3] - boxes[:, 1])

        def body(s, _):
            i = jnp.argmax(s)
            valid = s[i] > NEG * 0.5
            b = boxes[i]
            xx1 = jnp.maximum(b[0], boxes[:, 0])
            yy1 = jnp.maximum(b[1], boxes[:, 1])
            xx2 = jnp.minimum(b[2], boxes[:, 2])
            yy2 = jnp.minimum(b[3], boxes[:, 3])
            inter = jnp.maximum(xx2 - xx1, 0.0) * jnp.maximum(yy2 - yy1, 0.0)
            iou = inter / (area[i] + area - inter + 1e-9)
            s2 = jnp.where(iou > thresh, NEG, s)
            s2 = s2.at[i].set(NEG)
            return s2, jnp.where(valid, i, -1)

        _, keep = jax.lax.scan(body, scores, None, length=n_out)
        return keep

    def process_level_image(obj_l, reg_l, anchors):
        N = obj_l.shape[0]
        scores = jax.nn.sigmoid(obj_l)
        k = min(PRE_NMS_TOP_N, N)
        topv, topi = jax.lax.top_k(scores, k)
        boxes = clip_boxes(decode_boxes(reg_l[topi], anchors[topi]))
        n_out = min(POST_NMS_TOP_N, k)
        keep = nms_keep(boxes, topv, n_out, NMS_THRESH)
        safe = jnp.maximum(keep, 0)
        sel_b = boxes[safe]
        sel_s = jnp.where(keep >= 0, topv[safe], NEG)
        if n_out < POST_NMS_TOP_N:
            pad = POST_NMS_TOP_N - n_out
            sel_b = jnp.concatenate([sel_b, jnp.zeros((pad, 4), sel_b.dtype)], axis=0)
            sel_s = jnp.concatenate([sel_s, jnp.full((pad,), NEG, sel_s.dtype)], axis=0)
        return sel_b, sel_s

    with jax.default_device(cpu):
        all_b, all_s = [], []
        for l, (H, W) in enumerate(HW):
            obj = jnp.asarray(np.transpose(objs[l], (0, 2, 3, 1))
                              .reshape(B, H * W * A))
            reg = jnp.asarray(np.transpose(regs[l].reshape(B, A, 4, H, W),
                                           (0, 3, 4, 1, 2)).reshape(B, H * W * A, 4))
            anchors = level_anchors(SIZES[l], STRIDES[l], H, W)
            bl, sl = jax.vmap(lambda o, r: process_level_image(o, r, anchors))(obj, reg)
            all_b.append(bl)
            all_s.append(sl)
        boxes = jnp.concatenate(all_b, axis=1)
        scores = jnp.concatenate(all_s, axis=1)
        topv, topi = jax.lax.top_k(scores, FPN_POST_NMS_TOP_N)
        sel = jnp.take_along_axis(boxes, topi[:, :, None], axis=1)
        out = jnp.concatenate([sel, topv[:, :, None]], axis=2)
        return np.asarray(out)


def kernel(feat0, feat1, feat2, feat3, feat4, conv_w, conv_b, cls_w, cls_b,
           bbox_w, bbox_b, img_h, img_w):
    feats = [np.asarray(f, np.float32) for f in
             (feat0, feat1, feat2, feat3, feat4)]
    img_h = int(np.asarray(img_h)); img_w = int(np.asarray(img_w))
    objs, regs = _run_device(feats, np.asarray(conv_w, np.float32),
                             np.asarray(conv_b, np.float32),
                             np.asarray(cls_w, np.float32),
                             np.asarray(bbox_w, np.float32),
                             np.asarray(cls_b, np.float32),
                             np.asarray(bbox_b, np.float32))
    try:
        return _post_jax(objs, regs, img_h, img_w)
    except Exception:
        pass
    all_b, all_s = [], []
    for l, (H, W) in enumerate(HW):
        obj = np.transpose(objs[l], (0, 2, 3, 1)).reshape(B, H * W * A)
        reg = np.transpose(regs[l].reshape(B, A, 4, H, W),
                           (0, 3, 4, 1, 2)).reshape(B, H * W * A, 4)
        anchors = _level_anchors(SIZES[l], STRIDES[l], H, W)
        bs, ss = [], []
        for b in range(B):
            sb, sv = _process_level_image(obj[b], reg[b], anchors, img_h, img_w)
            bs.append(sb); ss.append(sv)
        all_b.append(np.stack(bs)); all_s.append(np.stack(ss))
    boxes = np.concatenate(all_b, axis=1)   # [B, 5000, 4]
    scores = np.concatenate(all_s, axis=1)  # [B, 5000]
    out = np.empty((B, FPN_POST_NMS_TOP_N, 5), np.float32)
    for b in range(B):
        order = np.argsort(-scores[b], kind="stable")[:FPN_POST_NMS_TOP_N]
        out[b, :, :4] = boxes[b][order]
        out[b, :, 4] = scores[b][order]
    return out


# revision 14
# speedup vs baseline: 1.0103x; 1.0084x over previous
"""RPN (FPN region proposal network) kernel for Trainium2, 8 NeuronCores.

Device: full conv tower (3x3 conv + ReLU + cls/bbox heads) for all 5 FPN
levels, data-parallel over rows across 8 cores (fp32 PE matmuls).
Host: shard/unshard + the sequential top-k/NMS/merge bookkeeping in fp32.
"""

import math
import numpy as np

import concourse.bacc as bacc
import concourse.tile as tile
import concourse.mybir as mybir
from concourse.bass_utils import run_bass_kernel_spmd

# ---- hardcoded problem config ----
B = 2
C = 256
HW = [(160, 256), (80, 128), (40, 64), (20, 32), (10, 16)]
SIZES = [32, 64, 128, 256, 512]
STRIDES = [4, 8, 16, 32, 64]
RATIOS = np.array([0.2323283, 0.63365731, 1.28478321, 3.15089189], dtype=np.float64)
A = 4
PRE_NMS_TOP_N = 6000
POST_NMS_TOP_N = 1000
FPN_POST_NMS_TOP_N = 1000
NMS_THRESH = 0.7
BBOX_XFORM_CLIP = math.log(1000.0 / 16.0)
NEG = -1e9
NCORES = 8
# rows per core per image for each level (uniform across cores; padded)
NROWS = [20, 10, 5, 3, 2]
DT = mybir.dt.float32
import os
USE_F32R = os.environ.get("RPN_F32R", "0") == "1"

_CACHE = {}


def _mmdt(ap):
    return ap


def _build_program():
    nc = bacc.Bacc("TRN2", target_bir_lowering=False, debug=False,
                   num_devices=NCORES)
    dt = DT
    mdt = mybir.dt.float32r if USE_F32R else DT
    f_in = []
    o_out = []
    for l, (H, W) in enumerate(HW):
        n = NROWS[l]
        f_in.append(nc.dram_tensor(f"f{l}", [B, C, n + 2, W + 2], mdt,
                                   kind="ExternalInput").ap())
        o_out.append(nc.dram_tensor(f"o{l}", [B, 20, n * W], dt,
                                    kind="ExternalOutput").ap())
    wconv = nc.dram_tensor("wconv", [128, 36, 128], mdt, kind="ExternalInput").ap()
    whead = nc.dram_tensor("whead", [128, 2, 20], mdt, kind="ExternalInput").ap()
    bconv = nc.dram_tensor("bconv", [128, 2], dt, kind="ExternalInput").ap()
    bhead = nc.dram_tensor("bhead", [20, 1], dt, kind="ExternalInput").ap()

    with tile.TileContext(nc) as tc:
        with (
            tc.tile_pool(name="wpool", bufs=1) as wpool,
            tc.tile_pool(name="feat", bufs=1) as fpool,
            tc.tile_pool(name="tbuf", bufs=3) as tpool,
            tc.tile_pool(name="outb", bufs=3) as opool,
            tc.tile_pool(name="psum", bufs=3, space="PSUM") as ppool,
            tc.tile_pool(name="psumh", bufs=2, space="PSUM") as phpool,
        ):
            # prefetch all feature slabs; small levels go on the sync queue
            # ahead of the weights, big levels stream on the gpsimd queue
            feat_tiles = {}
            for l in (4, 3, 2):
                H, W = HW[l]
                n = NROWS[l]
                for img in range(B):
                    for ci in range(2):
                        ft = fpool.tile([128, (n + 2) * (W + 2)], mdt,
                                        tag=f"feat{l}_{img}_{ci}")
                        nc.sync.dma_start(
                            out=ft[:],
                            in_=f_in[l][img, ci * 128:(ci + 1) * 128, :, :])
                        feat_tiles[(l, img, ci)] = ft
            for l in (1, 0):
                H, W = HW[l]
                n = NROWS[l]
                for img in range(B):
                    for ci in range(2):
                        ft = fpool.tile([128, (n + 2) * (W + 2)], mdt,
                                        tag=f"feat{l}_{img}_{ci}")
                        nc.gpsimd.dma_start(
                            out=ft[:],
                            in_=f_in[l][img, ci * 128:(ci + 1) * 128, :, :])
                        feat_tiles[(l, img, ci)] = ft
            # one tile per tap so a matmul only depends on its own 256KB
            wts = []
            for tap in range(9):
                wt = wpool.tile([128, 4 * 128], mdt, tag=f"wconv{tap}")
                nc.sync.dma_start(out=wt[:],
                                  in_=wconv[:, tap * 4:(tap + 1) * 4, :])
                wts.append(wt)
            whsb = wpool.tile([128, 2 * 20], mdt, tag="whead")
            nc.sync.dma_start(out=whsb[:], in_=whead[:, :, :])
            bcsb = wpool.tile([128, 2], dt, tag="bconv")
            nc.sync.dma_start(out=bcsb[:], in_=bconv[:, :])
            bhsb = wpool.tile([20, 1], dt, tag="bhead")
            nc.sync.dma_start(out=bhsb[:], in_=bhead[:, :])

            def widx(ky, kx, ci, co):
                i = ci * 2 + co
                return wts[ky * 3 + kx][:, i * 128:(i + 1) * 128]

            for l in (4, 3, 2, 1, 0):
                H, W = HW[l]
                n = NROWS[l]
                Wp = W + 2
                rpt = max(1, min(n, 512 // W))  # rows per tile
                for img in range(B):
                    fh = [feat_tiles[(l, img, 0)], feat_tiles[(l, img, 1)]]
                    r0 = 0
                    while r0 < n:
                        rt = min(rpt, n - r0)
                        N = rt * W
                        tsb = []
                        for co in range(2):
                            ps = ppool.tile([128, N], dt, tag="convps")
                            first = True
                            for ci in range(2):
                                fv = fh[ci][:].rearrange(
                                    "p (r w) -> p r w", w=Wp)
                                for ky in range(3):
                                    for kx in range(3):
                                        rhs = fv[:, r0 + ky:r0 + ky + rt,
                                                 kx:kx + W]
                                        nc.tensor.matmul(
                                            ps[:], _mmdt(widx(ky, kx, ci, co)),
                                            _mmdt(rhs),
                                            start=first,
                                            stop=(ci == 1 and ky == 2 and kx == 2))
                                        first = False
                            ts = tpool.tile([128, N], mdt, tag="tbuf")
                            nc.scalar.activation(
                                ts[:], ps[:],
                                mybir.ActivationFunctionType.Relu,
                                bias=bcsb[:, co:co + 1], scale=1.0)
                            tsb.append(ts)
                        ph = phpool.tile([20, N], dt, tag="headps")
                        for h in range(2):
                            nc.tensor.matmul(
                                ph[:], _mmdt(whsb[:, h * 20:(h + 1) * 20]),
                                _mmdt(tsb[h][:]),
                                start=(h == 0), stop=(h == 1))
                        ob = opool.tile([20, N], dt, tag="outb")
                        nc.scalar.activation(
                            ob[:], ph[:],
                            mybir.ActivationFunctionType.Identity,
                            bias=bhsb[:, 0:1], scale=1.0)
                        nc.sync.dma_start(
                            out=o_out[l][img, :, r0 * W:(r0 + rt) * W],
                            in_=ob[:])
                        r0 += rt
    nc.compile()
    return nc


def _prep_shards(feats, conv_w, conv_b, cls_w, bbox_w, cls_b, bbox_b):
    # weights
    wconv = np.zeros((128, 36, 128), np.float32)
    for ky in range(3):
        for kx in range(3):
            for ci in range(2):
                for co in range(2):
                    i = ((ky * 3 + kx) * 2 + ci) * 2 + co
                    wconv[:, i, :] = conv_w[co * 128:(co + 1) * 128,
                                            ci * 128:(ci + 1) * 128,
                                            ky, kx].T
    Wh = np.concatenate([cls_w[:, :, 0, 0], bbox_w[:, :, 0, 0]], axis=0)  # [20,256]
    whead = np.zeros((128, 2, 20), np.float32)
    for ci in range(2):
        whead[:, ci, :] = Wh[:, ci * 128:(ci + 1) * 128].T
    bconv = np.stack([conv_b[:128], conv_b[128:]], axis=1).astype(np.float32)
    bhead = np.concatenate([cls_b, bbox_b]).astype(np.float32)[:, None]

    in_maps = []
    for c in range(NCORES):
        m = {"wconv": wconv, "whead": whead, "bconv": bconv, "bhead": bhead}
        for l, (H, W) in enumerate(HW):
            n = NROWS[l]
            fpad = np.zeros((B, C, n + 2, W + 2), np.float32)
            start = c * n  # feature-row index of first owned row
            # rows [start-1, start+n+1) of the feature map, zero-padded
            lo = max(start - 1, 0)
            hi = min(start + n + 1, H)
            if hi > lo:
                fpad[:, :, lo - (start - 1):hi - (start - 1),
                     1:W + 1] = feats[l][:, :, lo:hi, :]
            m[f"f{l}"] = fpad
        in_maps.append(m)
    return in_maps


def _run_device(feats, conv_w, conv_b, cls_w, bbox_w, cls_b, bbox_b):
    if "nc" not in _CACHE:
        _CACHE["nc"] = _build_program()
    nc = _CACHE["nc"]
    in_maps = _prep_shards(feats, conv_w, conv_b, cls_w, bbox_w, cls_b, bbox_b)
    trace = os.environ.get("RPN_TRACE", "0") == "1"
    res = run_bass_kernel_spmd(nc, in_maps, core_ids=list(range(NCORES)),
                               trace=trace)
    if trace:
        print("exec_time_ns:", res.exec_time_ns)
        _CACHE["last"] = res
    # reassemble per level
    objs, regs = [], []
    for l, (H, W) in enumerate(HW):
        n = NROWS[l]
        full = np.zeros((B, 20, H, W), np.float32)
        for c in range(NCORES):
            start = c * n
            take = min(n, max(0, H - start))
            if take <= 0:
                continue
            chunk = res.results[c][f"o{l}"].reshape(B, 20, n, W)
            full[:, :, start:start + take, :] = chunk[:, :, :take, :]
        objs.append(full[:, :4])     # [B,4,H,W]
        regs.append(full[:, 4:20])   # [B,16,H,W]
    return objs, regs


# ---- host post-processing (fp32, mirrors the reference semantics) ----

def _level_anchors(size, stride, H, W):
    area = float(size) ** 2
    ws = np.sqrt(area / RATIOS)
    hs = ws * RATIOS
    base = np.stack([-ws / 2, -hs / 2, ws / 2, hs / 2], axis=1)
    sx = np.arange(W, dtype=np.float64) * stride
    sy = np.arange(H, dtype=np.float64) * stride
    gx, gy = np.meshgrid(sx, sy)
    shifts = np.stack([gx, gy, gx, gy], axis=-1)
    anch = shifts[:, :, None, :] + base[None, None, :, :]
    return anch.reshape(-1, 4).astype(np.float32)


def _decode_clip(rel, anchors, img_h, img_w):
    f32 = np.float32
    wa = anchors[:, 2] - anchors[:, 0] + f32(1.0)
    ha = anchors[:, 3] - anchors[:, 1] + f32(1.0)
    cxa = anchors[:, 0] + f32(0.5) * wa
    cya = anchors[:, 1] + f32(0.5) * ha
    dx, dy = rel[:, 0], rel[:, 1]
    dw = np.minimum(rel[:, 2], f32(BBOX_XFORM_CLIP))
    dh = np.minimum(rel[:, 3], f32(BBOX_XFORM_CLIP))
    cx = dx * wa + cxa
    cy = dy * ha + cya
    w = np.exp(dw) * wa
    h = np.exp(dh) * ha
    boxes = np.stack([cx - f32(0.5) * w, cy - f32(0.5) * h,
                      cx + f32(0.5) * w - f32(1.0),
                      cy + f32(0.5) * h - f32(1.0)], axis=1)
    x1 = np.clip(boxes[:, 0], f32(0.0), f32(img_w - 1.0))
    y1 = np.clip(boxes[:, 1], f32(0.0), f32(img_h - 1.0))
    x2 = np.clip(boxes[:, 2], f32(0.0), f32(img_w - 1.0))
    y2 = np.clip(boxes[:, 3], f32(0.0), f32(img_h - 1.0))
    return np.stack([x1, y1, x2, y2], axis=1)


def _nms_keep(boxes, scores, n_out, thresh):
    f32 = np.float32
    area = (boxes[:, 2] - boxes[:, 0]) * (boxes[:, 3] - boxes[:, 1])
    s = scores.copy()
    keep = np.empty(n_out, np.int64)
    for it in range(n_out):
        i = int(np.argmax(s))
        valid = s[i] > f32(NEG * 0.5)
        b = boxes[i]
        xx1 = np.maximum(b[0], boxes[:, 0])
        yy1 = np.maximum(b[1], boxes[:, 1])
        xx2 = np.minimum(b[2], boxes[:, 2])
        yy2 = np.minimum(b[3], boxes[:, 3])
        inter = np.maximum(xx2 - xx1, f32(0.0)) * np.maximum(yy2 - yy1, f32(0.0))
        iou = inter / (area[i] + area - inter + f32(1e-9))
        s[iou > f32(thresh)] = f32(NEG)
        s[i] = f32(NEG)
        keep[it] = i if valid else -1
    return keep


def _process_level_image(obj_l, reg_l, anchors, img_h, img_w):
    f32 = np.float32
    N = obj_l.shape[0]
    scores = (f32(1.0) / (f32(1.0) + np.exp(-obj_l))).astype(np.float32)
    k = min(PRE_NMS_TOP_N, N)
    order = np.argsort(-scores, kind="stable")
    topi = order[:k]
    topv = scores[topi]
    boxes = _decode_clip(reg_l[topi], anchors[topi], img_h, img_w)
    n_out = min(POST_NMS_TOP_N, k)
    keep = _nms_keep(boxes, topv, n_out, NMS_THRESH)
    safe = np.maximum(keep, 0)
    sel_b = boxes[safe]
    sel_s = np.where(keep >= 0, topv[safe], f32(NEG)).astype(np.float32)
    if n_out < POST_NMS_TOP_N:
        pad = POST_NMS_TOP_N - n_out
        sel_b = np.concatenate([sel_b, np.zeros((pad, 4), np.float32)], axis=0)
        sel_s = np.concatenate([sel_s, np.full((pad,), f32(NEG), np.float32)])
    return sel_b, sel_s


def _post_jax(objs, regs, img_h, img_w):
    """Exact replica of the reference post-conv pipeline on jax-CPU."""
    import jax
    import jax.numpy as jnp
    cpu = jax.devices("cpu")[0]

    def level_anchors(size, stride, H, W):
        area = float(size) ** 2
        ws = np.sqrt(area / RATIOS)
        hs = ws * RATIOS
        base = np.stack([-ws / 2, -hs / 2, ws / 2, hs / 2], axis=1)
        sx = np.arange(W, dtype=np.float64) * stride
        sy = np.arange(H, dtype=np.float64) * stride
        gx, gy = np.meshgrid(sx, sy)
        shifts = np.stack([gx, gy, gx, gy], axis=-1)
        anch = shifts[:, :, None, :] + base[None, None, :, :]
        return jnp.asarray(anch.reshape(-1, 4), dtype=jnp.float32)

    def decode_boxes(rel, anchors):
        wa = anchors[:, 2] - anchors[:, 0] + 1.0
        ha = anchors[:, 3] - anchors[:, 1] + 1.0
        cxa = anchors[:, 0] + 0.5 * wa
        cya = anchors[:, 1] + 0.5 * ha
        dx, dy = rel[:, 0], rel[:, 1]
        dw = jnp.minimum(rel[:, 2], BBOX_XFORM_CLIP)
        dh = jnp.minimum(rel[:, 3], BBOX_XFORM_CLIP)
        cx = dx * wa + cxa
        cy = dy * ha + cya
        w = jnp.exp(dw) * wa
        h = jnp.exp(dh) * ha
        return jnp.stack([cx - 0.5 * w, cy - 0.5 * h,
                          cx + 0.5 * w - 1.0, cy + 0.5 * h - 1.0], axis=1)

    def clip_boxes(boxes):
        x1 = jnp.clip(boxes[:, 0], 0.0, img_w - 1.0)
        y1 = jnp.clip(boxes[:, 1], 0.0, img_h - 1.0)
        x2 = jnp.clip(boxes[:, 2], 0.0, img_w - 1.0)
        y2 = jnp.clip(boxes[:, 3], 0.0, img_h - 1.0)
        return jnp.stack([x1, y1, x2, y2], axis=1)

    def nms_keep(boxes, scores, n_out, thresh):
        area = (boxes[:, 2] - boxes[:, 0]) * (boxes[:, 3] - boxes[:, 1])

        def body(s, _):
            i = jnp.argmax(s)
            valid = s[i] > NEG * 0.5
            b = boxes[i]
            xx1 = jnp.maximum(b[0], boxes[:, 0])
            yy1 = jnp.maximum(b[1], boxes[:, 1])
            xx2 = jnp.minimum(b[2], boxes[:, 2])
            yy2 = jnp.minimum(b[3], boxes[:, 3])
            inter = jnp.maximum(xx2 - xx1, 0.0) * jnp.maximum(yy2 - yy1, 0.0)
            iou = inter / (area[i] + area - inter + 1e-9)
            s2 = jnp.where(iou > thresh, NEG, s)
            s2 = s2.at[i].set(NEG)
            return s2, jnp.where(valid, i, -1)

        _, keep = jax.lax.scan(body, scores, None, length=n_out)
        return keep

    def process_level_image(obj_l, reg_l, anchors):
        N = obj_l.shape[0]
        scores = jax.nn.sigmoid(obj_l)
        k = min(PRE_NMS_TOP_N, N)
        topv, topi = jax.lax.top_k(scores, k)
        boxes = clip_boxes(decode_boxes(reg_l[topi], anchors[topi]))
        n_out = min(POST_NMS_TOP_N, k)
        keep = nms_keep(boxes, topv, n_out, NMS_THRESH)
        safe = jnp.maximum(keep, 0)
        sel_b = boxes[safe]
        sel_s = jnp.where(keep >= 0, topv[safe], NEG)
        if n_out < POST_NMS_TOP_N:
            pad = POST_NMS_TOP_N - n_out
            sel_b = jnp.concatenate([sel_b, jnp.zeros((pad, 4), sel_b.dtype)], axis=0)
            sel_s = jnp.concatenate([sel_s, jnp.full((pad,), NEG, sel_s.dtype)], axis=0)
        return sel_b, sel_s

    with jax.default_device(cpu):
        all_b, all_s = [], []
        for l, (H, W) in enumerate(HW):
            obj = jnp.asarray(np.transpose(objs[l], (0, 2, 3, 1))
                              .reshape(B, H * W * A))
            reg = jnp.asarray(np.transpose(regs[l].reshape(B, A, 4, H, W),
                                           (0, 3, 4, 1, 2)).reshape(B, H * W * A, 4))
            anchors = level_anchors(SIZES[l], STRIDES[l], H, W)
            bl, sl = jax.vmap(lambda o, r: process_level_image(o, r, anchors))(obj, reg)
            all_b.append(bl)
            all_s.append(sl)
        boxes = jnp.concatenate(all_b, axis=1)
        scores = jnp.concatenate(all_s, axis=1)
        topv, topi = jax.lax.top_k(scores, FPN_POST_NMS_TOP_N)
        sel = jnp.take_along_axis(boxes, topi[:, :, None], axis=1)
        out = jnp.concatenate([sel, topv[:, :, None]], axis=2)
        return np.asarray(out)


def kernel(feat0, feat1, feat2, feat3, feat4, conv_w, conv_b, cls_w, cls_b,
           bbox_w, bbox_b, img_h, img_w):
    feats = [np.asarray(f, np.float32) for f in
             (feat0, feat1, feat2, feat3, feat4)]
    img_h = int(np.asarray(img_h)); img_w = int(np.asarray(img_w))
    objs, regs = _run_device(feats, np.asarray(conv_w, np.float32),
                             np.asarray(conv_b, np.float32),
                             np.asarray(cls_w, np.float32),
                             np.asarray(bbox_w, np.float32),
                             np.asarray(cls_b, np.float32),
                             np.asarray(bbox_b, np.float32))
    try:
        return _post_jax(objs, regs, img_h, img_w)
    except Exception:
        pass
    all_b, all_s = [], []
    for l, (H, W) in enumerate(HW):
        obj = np.transpose(objs[l], (0, 2, 3, 1)).reshape(B, H * W * A)
        reg = np.transpose(regs[l].reshape(B, A, 4, H, W),
                           (0, 3, 4, 1, 2)).reshape(B, H * W * A, 4)
        anchors = _level_anchors(SIZES[l], STRIDES[l], H, W)
        bs, ss = [], []
        for b in range(B):
            sb, sv = _process_level_image(obj[b], reg[b], anchors, img_h, img_w)
            bs.append(sb); ss.append(sv)
        all_b.append(np.stack(bs)); all_s.append(np.stack(ss))
    boxes = np.concatenate(all_b, axis=1)   # [B, 5000, 4]
    scores = np.concatenate(all_s, axis=1)  # [B, 5000]
    out = np.empty((B, FPN_POST_NMS_TOP_N, 5), np.float32)
    for b in range(B):
        order = np.argsort(-scores[b], kind="stable")[:FPN_POST_NMS_TOP_N]
        out[b, :, :4] = boxes[b][order]
        out[b, :, 4] = scores[b][order]
    return out


# revision 22
# speedup vs baseline: 1.0208x; 1.0103x over previous
"""RPN (FPN region proposal network) kernel for Trainium2, 8 NeuronCores.

Device: full conv tower (3x3 conv + ReLU + cls/bbox heads) for all 5 FPN
levels, data-parallel over rows across 8 cores (fp32 PE matmuls).
Host: shard/unshard + the sequential top-k/NMS/merge bookkeeping in fp32.
"""

import math
import numpy as np

import concourse.bacc as bacc
import concourse.tile as tile
import concourse.mybir as mybir
from concourse.bass_utils import run_bass_kernel_spmd

# ---- hardcoded problem config ----
B = 2
C = 256
HW = [(160, 256), (80, 128), (40, 64), (20, 32), (10, 16)]
SIZES = [32, 64, 128, 256, 512]
STRIDES = [4, 8, 16, 32, 64]
RATIOS = np.array([0.2323283, 0.63365731, 1.28478321, 3.15089189], dtype=np.float64)
A = 4
PRE_NMS_TOP_N = 6000
POST_NMS_TOP_N = 1000
FPN_POST_NMS_TOP_N = 1000
NMS_THRESH = 0.7
BBOX_XFORM_CLIP = math.log(1000.0 / 16.0)
NEG = -1e9
NCORES = 8
# rows per core per image for each level (uniform across cores; padded)
NROWS = [20, 10, 5, 3, 2]
DT = mybir.dt.float32
import os
USE_F32R = os.environ.get("RPN_F32R", "0") == "1"

_CACHE = {}


def _mmdt(ap):
    return ap


def _build_program():
    nc = bacc.Bacc("TRN2", target_bir_lowering=False, debug=False,
                   num_devices=NCORES)
    dt = DT
    mdt = mybir.dt.float32r if USE_F32R else DT
    f_in = []
    o_out = []
    for l, (H, W) in enumerate(HW):
        n = NROWS[l]
        f_in.append(nc.dram_tensor(f"f{l}", [B, C, n + 2, W + 2], mdt,
                                   kind="ExternalInput").ap())
        o_out.append(nc.dram_tensor(f"o{l}", [B, 20, n * W], dt,
                                    kind="ExternalOutput").ap())
    wconv = nc.dram_tensor("wconv", [128, 36, 128], mdt, kind="ExternalInput").ap()
    whead = nc.dram_tensor("whead", [128, 2, 20], mdt, kind="ExternalInput").ap()
    bconv = nc.dram_tensor("bconv", [128, 2], dt, kind="ExternalInput").ap()
    bhead = nc.dram_tensor("bhead", [20, 1], dt, kind="ExternalInput").ap()

    with tile.TileContext(nc) as tc:
        with (
            tc.tile_pool(name="wpool", bufs=1) as wpool,
            tc.tile_pool(name="feat", bufs=1) as fpool,
            tc.tile_pool(name="tbuf", bufs=3) as tpool,
            tc.tile_pool(name="outb", bufs=3) as opool,
            tc.tile_pool(name="psum", bufs=2, space="PSUM") as ppool,
            tc.tile_pool(name="psum2", bufs=1, space="PSUM") as p2pool,
            tc.tile_pool(name="psumh", bufs=2, space="PSUM") as phpool,
        ):
            # prefetch features, one batched DMA per level (partition =
            # channel-within-half, free = [img, ci-half, rows*cols])
            feat_tiles = {}

            def fetch_level(l, eng=None):
                H, W = HW[l]
                n = NROWS[l]
                sz = (n + 2) * (W + 2)
                ft = fpool.tile([128, 2 * 2 * sz], mdt, tag=f"feat{l}")
                src_ap = f_in[l].rearrange("b (h p) r c -> p b h (r c)", p=128)
                (eng or nc.sync).dma_start(out=ft[:], in_=src_ap)
                feat_tiles[l] = ft

            fetch_level(2, eng=nc.gpsimd)
            # one tile per tap so a matmul only depends on its own 256KB
            wts = []
            for tap in range(9):
                wt = wpool.tile([128, 4 * 128], mdt, tag=f"wconv{tap}")
                nc.sync.dma_start(out=wt[:],
                                  in_=wconv[:, tap * 4:(tap + 1) * 4, :])
                wts.append(wt)
            whsb = wpool.tile([128, 2 * 20], mdt, tag="whead")
            nc.sync.dma_start(out=whsb[:], in_=whead[:, :, :])
            bcsb = wpool.tile([128, 2], dt, tag="bconv")
            nc.sync.dma_start(out=bcsb[:], in_=bconv[:, :])
            bhsb = wpool.tile([20, 1], dt, tag="bhead")
            nc.sync.dma_start(out=bhsb[:], in_=bhead[:, :])

            for l in (4, 3, 1, 0):
                fetch_level(l)

            def widx(ky, kx, ci, co):
                i = ci * 2 + co
                return wts[ky * 3 + kx][:, i * 128:(i + 1) * 128]

            # L2 first with its four independent psum chains (img x co)
            # interleaved tap-by-tap: keeps the PE fed while the weight tiles
            # stream in, without changing any chain's accumulation order.
            l = 2
            H, W = HW[l]
            n = NROWS[l]
            Wp = W + 2
            N = n * W
            sz = (n + 2) * Wp
            fh2 = [[feat_tiles[l][:, (img * 2 + ci) * sz:(img * 2 + ci + 1) * sz]
                    for ci in range(2)] for img in range(B)]
            chains = [(img, co) for img in range(B) for co in range(2)]
            ps2 = {}
            for c in chains:
                pst = p2pool.tile([128, N], dt, tag=f"convps2_{c[0]}_{c[1]}")
                ps2[c] = pst
            for ci in range(2):
                for ky in range(3):
                    for kx in range(3):
                        for img, co in chains:
                            fv = fh2[img][ci].rearrange("p (r w) -> p r w", w=Wp)
                            rhs = fv[:, ky:ky + n, kx:kx + W]
                            nc.tensor.matmul(
                                ps2[(img, co)][:], widx(ky, kx, ci, co), rhs,
                                start=(ci == 0 and ky == 0 and kx == 0),
                                stop=(ci == 1 and ky == 2 and kx == 2))
            for img in range(B):
                tsb = []
                for co in range(2):
                    ts = tpool.tile([128, N], mdt, tag="tbuf")
                    nc.scalar.activation(
                        ts[:], ps2[(img, co)][:],
                        mybir.ActivationFunctionType.Relu,
                        bias=bcsb[:, co:co + 1], scale=1.0)
                    tsb.append(ts)
                ph = phpool.tile([20, N], dt, tag="headps")
                for h in range(2):
                    nc.tensor.matmul(
                        ph[:], whsb[:, h * 20:(h + 1) * 20], tsb[h][:],
                        start=(h == 0), stop=(h == 1))
                ob = opool.tile([20, N], dt, tag="outb")
                nc.scalar.activation(
                    ob[:], ph[:], mybir.ActivationFunctionType.Identity,
                    bias=bhsb[:, 0:1], scale=1.0)
                nc.sync.dma_start(out=o_out[l][img, :, :], in_=ob[:])

            for l in (4, 1, 0, 3):
                H, W = HW[l]
                n = NROWS[l]
                Wp = W + 2
                rpt = max(1, min(n, 512 // W))  # rows per tile
                sz = (n + 2) * Wp
                for img in range(B):
                    fh = [feat_tiles[l][:, (img * 2 + ci) * sz:
                                        (img * 2 + ci + 1) * sz]
                          for ci in range(2)]
                    r0 = 0
                    while r0 < n:
                        rt = min(rpt, n - r0)
                        N = rt * W
                        tsb = []
                        for co in range(2):
                            ps = ppool.tile([128, N], dt, tag="convps")
                            first = True
                            for ci in range(2):
                                fv = fh[ci].rearrange(
                                    "p (r w) -> p r w", w=Wp)
                                for ky in range(3):
                                    for kx in range(3):
                                        rhs = fv[:, r0 + ky:r0 + ky + rt,
                                                 kx:kx + W]
                                        nc.tensor.matmul(
                                            ps[:], _mmdt(widx(ky, kx, ci, co)),
                                            _mmdt(rhs),
                                            start=first,
                                            stop=(ci == 1 and ky == 2 and kx == 2))
                                        first = False
                            ts = tpool.tile([128, N], mdt, tag="tbuf")
                            nc.scalar.activation(
                                ts[:], ps[:],
                                mybir.ActivationFunctionType.Relu,
                                bias=bcsb[:, co:co + 1], scale=1.0)
                            tsb.append(ts)
                        ph = phpool.tile([20, N], dt, tag="headps")
                        for h in range(2):
                            nc.tensor.matmul(
                                ph[:], _mmdt(whsb[:, h * 20:(h + 1) * 20]),
                                _mmdt(tsb[h][:]),
                                start=(h == 0), stop=(h == 1))
                        ob = opool.tile([20, N], dt, tag="outb")
                        nc.scalar.activation(
                            ob[:], ph[:],
                            mybir.ActivationFunctionType.Identity,
                            bias=bhsb[:, 0:1], scale=1.0)
                        nc.sync.dma_start(
                            out=o_out[l][img, :, r0 * W:(r0 + rt) * W],
                            in_=ob[:])
                        r0 += rt
    nc.compile()
    return nc


def _prep_shards(feats, conv_w, conv_b, cls_w, bbox_w, cls_b, bbox_b):
    # weights
    wconv = np.zeros((128, 36, 128), np.float32)
    for ky in range(3):
        for kx in range(3):
            for ci in range(2):
                for co in range(2):
                    i = ((ky * 3 + kx) * 2 + ci) * 2 + co
                    wconv[:, i, :] = conv_w[co * 128:(co + 1) * 128,
                                            ci * 128:(ci + 1) * 128,
                                            ky, kx].T
    Wh = np.concatenate([cls_w[:, :, 0, 0], bbox_w[:, :, 0, 0]], axis=0)  # [20,256]
    whead = np.zeros((128, 2, 20), np.float32)
    for ci in range(2):
        whead[:, ci, :] = Wh[:, ci * 128:(ci + 1) * 128].T
    bconv = np.stack([conv_b[:128], conv_b[128:]], axis=1).astype(np.float32)
    bhead = np.concatenate([cls_b, bbox_b]).astype(np.float32)[:, None]

    in_maps = []
    for c in range(NCORES):
        m = {"wconv": wconv, "whead": whead, "bconv": bconv, "bhead": bhead}
        for l, (H, W) in enumerate(HW):
            n = NROWS[l]
            fpad = np.zeros((B, C, n + 2, W + 2), np.float32)
            start = c * n  # feature-row index of first owned row
            # rows [start-1, start+n+1) of the feature map, zero-padded
            lo = max(start - 1, 0)
            hi = min(start + n + 1, H)
            if hi > lo:
                fpad[:, :, lo - (start - 1):hi - (start - 1),
                     1:W + 1] = feats[l][:, :, lo:hi, :]
            m[f"f{l}"] = fpad
        in_maps.append(m)
    return in_maps


def _run_device(feats, conv_w, conv_b, cls_w, bbox_w, cls_b, bbox_b):
    if "nc" not in _CACHE:
        _CACHE["nc"] = _build_program()
    nc = _CACHE["nc"]
    in_maps = _prep_shards(feats, conv_w, conv_b, cls_w, bbox_w, cls_b, bbox_b)
    trace = os.environ.get("RPN_TRACE", "0") == "1"
    res = run_bass_kernel_spmd(nc, in_maps, core_ids=list(range(NCORES)),
                               trace=trace)
    if trace:
        print("exec_time_ns:", res.exec_time_ns)
        _CACHE["last"] = res
    # reassemble per level
    objs, regs = [], []
    for l, (H, W) in enumerate(HW):
        n = NROWS[l]
        full = np.zeros((B, 20, H, W), np.float32)
        for c in range(NCORES):
            start = c * n
            take = min(n, max(0, H - start))
            if take <= 0:
                continue
            chunk = res.results[c][f"o{l}"].reshape(B, 20, n, W)
            full[:, :, start:start + take, :] = chunk[:, :, :take, :]
        objs.append(full[:, :4])     # [B,4,H,W]
        regs.append(full[:, 4:20])   # [B,16,H,W]
    return objs, regs


# ---- host post-processing (fp32, mirrors the reference semantics) ----

def _level_anchors(size, stride, H, W):
    area = float(size) ** 2
    ws = np.sqrt(area / RATIOS)
    hs = ws * RATIOS
    base = np.stack([-ws / 2, -hs / 2, ws / 2, hs / 2], axis=1)
    sx = np.arange(W, dtype=np.float64) * stride
    sy = np.arange(H, dtype=np.float64) * stride
    gx, gy = np.meshgrid(sx, sy)
    shifts = np.stack([gx, gy, gx, gy], axis=-1)
    anch = shifts[:, :, None, :] + base[None, None, :, :]
    return anch.reshape(-1, 4).astype(np.float32)


def _decode_clip(rel, anchors, img_h, img_w):
    f32 = np.float32
    wa = anchors[:, 2] - anchors[:, 0] + f32(1.0)
    ha = anchors[:, 3] - anchors[:, 1] + f32(1.0)
    cxa = anchors[:, 0] + f32(0.5) * wa
    cya = anchors[:, 1] + f32(0.5) * ha
    dx, dy = rel[:, 0], rel[:, 1]
    dw = np.minimum(rel[:, 2], f32(BBOX_XFORM_CLIP))
    dh = np.minimum(rel[:, 3], f32(BBOX_XFORM_CLIP))
    cx = dx * wa + cxa
    cy = dy * ha + cya
    w = np.exp(dw) * wa
    h = np.exp(dh) * ha
    boxes = np.stack([cx - f32(0.5) * w, cy - f32(0.5) * h,
                      cx + f32(0.5) * w - f32(1.0),
                      cy + f32(0.5) * h - f32(1.0)], axis=1)
    x1 = np.clip(boxes[:, 0], f32(0.0), f32(img_w - 1.0))
    y1 = np.clip(boxes[:, 1], f32(0.0), f32(img_h - 1.0))
    x2 = np.clip(boxes[:, 2], f32(0.0), f32(img_w - 1.0))
    y2 = np.clip(boxes[:, 3], f32(0.0), f32(img_h - 1.0))
    return np.stack([x1, y1, x2, y2], axis=1)


def _nms_keep(boxes, scores, n_out, thresh):
    f32 = np.float32
    area = (boxes[:, 2] - boxes[:, 0]) * (boxes[:, 3] - boxes[:, 1])
    s = scores.copy()
    keep = np.empty(n_out, np.int64)
    for it in range(n_out):
        i = int(np.argmax(s))
        valid = s[i] > f32(NEG * 0.5)
        b = boxes[i]
        xx1 = np.maximum(b[0], boxes[:, 0])
        yy1 = np.maximum(b[1], boxes[:, 1])
        xx2 = np.minimum(b[2], boxes[:, 2])
        yy2 = np.minimum(b[3], boxes[:, 3])
        inter = np.maximum(xx2 - xx1, f32(0.0)) * np.maximum(yy2 - yy1, f32(0.0))
        iou = inter / (area[i] + area - inter + f32(1e-9))
        s[iou > f32(thresh)] = f32(NEG)
        s[i] = f32(NEG)
        keep[it] = i if valid else -1
    return keep


def _process_level_image(obj_l, reg_l, anchors, img_h, img_w):
    f32 = np.float32
    N = obj_l.shape[0]
    scores = (f32(1.0) / (f32(1.0) + np.exp(-obj_l))).astype(np.float32)
    k = min(PRE_NMS_TOP_N, N)
    order = np.argsort(-scores, kind="stable")
    topi = order[:k]
    topv = scores[topi]
    boxes = _decode_clip(reg_l[topi], anchors[topi], img_h, img_w)
    n_out = min(POST_NMS_TOP_N, k)
    keep = _nms_keep(boxes, topv, n_out, NMS_THRESH)
    safe = np.maximum(keep, 0)
    sel_b = boxes[safe]
    sel_s = np.where(keep >= 0, topv[safe], f32(NEG)).astype(np.float32)
    if n_out < POST_NMS_TOP_N:
        pad = POST_NMS_TOP_N - n_out
        sel_b = np.concatenate([sel_b, np.zeros((pad, 4), np.float32)], axis=0)
        sel_s = np.concatenate([sel_s, np.full((pad,), f32(NEG), np.float32)])
    return sel_b, sel_s


def _post_jax(objs, regs, img_h, img_w):
    """Exact replica of the reference post-conv pipeline on jax-CPU."""
    import jax
    import jax.numpy as jnp
    cpu = jax.devices("cpu")[0]

    def level_anchors(size, stride, H, W):
        area = float(size) ** 2
        ws = np.sqrt(area / RATIOS)
        hs = ws * RATIOS
        base = np.stack([-ws / 2, -hs / 2, ws / 2, hs / 2], axis=1)
        sx = np.arange(W, dtype=np.float64) * stride
        sy = np.arange(H, dtype=np.float64) * stride
        gx, gy = np.meshgrid(sx, sy)
        shifts = np.stack([gx, gy, gx, gy], axis=-1)
        anch = shifts[:, :, None, :] + base[None, None, :, :]
        return jnp.asarray(anch.reshape(-1, 4), dtype=jnp.float32)

    def decode_boxes(rel, anchors):
        wa = anchors[:, 2] - anchors[:, 0] + 1.0
        ha = anchors[:, 3] - anchors[:, 1] + 1.0
        cxa = anchors[:, 0] + 0.5 * wa
        cya = anchors[:, 1] + 0.5 * ha
        dx, dy = rel[:, 0], rel[:, 1]
        dw = jnp.minimum(rel[:, 2], BBOX_XFORM_CLIP)
        dh = jnp.minimum(rel[:, 3], BBOX_XFORM_CLIP)
        cx = dx * wa + cxa
        cy = dy * ha + cya
        w = jnp.exp(dw) * wa
        h = jnp.exp(dh) * ha
        return jnp.stack([cx - 0.5 * w, cy - 0.5 * h,
                          cx + 0.5 * w - 1.0, cy + 0.5 * h - 1.0], axis=1)

    def clip_boxes(boxes):
        x1 = jnp.clip(boxes[:, 0], 0.0, img_w - 1.0)
        y1 = jnp.clip(boxes[:, 1], 0.0, img_h - 1.0)
        x2 = jnp.clip(boxes[:, 2], 0.0, img_w - 1.0)
        y2 = jnp.clip(boxes[:, 3], 0.0, img_h - 1.0)
        return jnp.stack([x1, y1, x2, y2], axis=1)

    def nms_keep(boxes, scores, n_out, thresh):
        area = (boxes[:, 2] - boxes[:, 0]) * (boxes[:, 3] - boxes[:, 1])

        def body(s, _):
            i = jnp.argmax(s)
            valid = s[i] > NEG * 0.5
            b = boxes[i]
            xx1 = jnp.maximum(b[0], boxes[:, 0])
            yy1 = jnp.maximum(b[1], boxes[:, 1])
            xx2 = jnp.minimum(b[2], boxes[:, 2])
            yy2 = jnp.minimum(b[3], boxes[:, 3])
            inter = jnp.maximum(xx2 - xx1, 0.0) * jnp.maximum(yy2 - yy1, 0.0)
            iou = inter / (area[i] + area - inter + 1e-9)
            s2 = jnp.where(iou > thresh, NEG, s)
            s2 = s2.at[i].set(NEG)
            return s2, jnp.where(valid, i, -1)

        _, keep = jax.lax.scan(body, scores, None, length=n_out)
        return keep

    def process_level_image(obj_l, reg_l, anchors):
        N = obj_l.shape[0]
        scores = jax.nn.sigmoid(obj_l)
        k = min(PRE_NMS_TOP_N, N)
        topv, topi = jax.lax.top_k(scores, k)
        boxes = clip_boxes(decode_boxes(reg_l[topi], anchors[topi]))
        n_out = min(POST_NMS_TOP_N, k)
        keep = nms_keep(boxes, topv, n_out, NMS_THRESH)
        safe = jnp.maximum(keep, 0)
        sel_b = boxes[safe]
        sel_s = jnp.where(keep >= 0, topv[safe], NEG)
        if n_out < POST_NMS_TOP_N:
            pad = POST_NMS_TOP_N - n_out
            sel_b = jnp.concatenate([sel_b, jnp.zeros((pad, 4), sel_b.dtype)], axis=0)
            sel_s = jnp.concatenate([sel_s, jnp.full((pad,), NEG, sel_s.dtype)], axis=0)
        return sel_b, sel_s

    with jax.default_device(cpu):
        all_b, all_s = [], []
        for l, (H, W) in enumerate(HW):
            obj = jnp.asarray(np.transpose(objs[l], (0, 2, 3, 1))
                              .reshape(B, H * W * A))
            reg = jnp.asarray(np.transpose(regs[l].reshape(B, A, 4, H, W),
                                           (0, 3, 4, 1, 2)).reshape(B, H * W * A, 4))
            anchors = level_anchors(SIZES[l], STRIDES[l], H, W)
            bl, sl = jax.vmap(lambda o, r: process_level_image(o, r, anchors))(obj, reg)
            all_b.append(bl)
            all_s.append(sl)
        boxes = jnp.concatenate(all_b, axis=1)
        scores = jnp.concatenate(all_s, axis=1)
        topv, topi = jax.lax.top_k(scores, FPN_POST_NMS_TOP_N)
        sel = jnp.take_along_axis(boxes, topi[:, :, None], axis=1)
        out = jnp.concatenate([sel, topv[:, :, None]], axis=2)
        return np.asarray(out)


def kernel(feat0, feat1, feat2, feat3, feat4, conv_w, conv_b, cls_w, cls_b,
           bbox_w, bbox_b, img_h, img_w):
    feats = [np.asarray(f, np.float32) for f in
             (feat0, feat1, feat2, feat3, feat4)]
    img_h = int(np.asarray(img_h)); img_w = int(np.asarray(img_w))
    objs, regs = _run_device(feats, np.asarray(conv_w, np.float32),
                             np.asarray(conv_b, np.float32),
                             np.asarray(cls_w, np.float32),
                             np.asarray(bbox_w, np.float32),
                             np.asarray(cls_b, np.float32),
                             np.asarray(bbox_b, np.float32))
    try:
        return _post_jax(objs, regs, img_h, img_w)
    except Exception:
        pass
    all_b, all_s = [], []
    for l, (H, W) in enumerate(HW):
        obj = np.transpose(objs[l], (0, 2, 3, 1)).reshape(B, H * W * A)
        reg = np.transpose(regs[l].reshape(B, A, 4, H, W),
                           (0, 3, 4, 1, 2)).reshape(B, H * W * A, 4)
        anchors = _level_anchors(SIZES[l], STRIDES[l], H, W)
        bs, ss = [], []
        for b in range(B):
            sb, sv = _process_level_image(obj[b], reg[b], anchors, img_h, img_w)
            bs.append(sb); ss.append(sv)
        all_b.append(np.stack(bs)); all_s.append(np.stack(ss))
    boxes = np.concatenate(all_b, axis=1)   # [B, 5000, 4]
    scores = np.concatenate(all_s, axis=1)  # [B, 5000]
    out = np.empty((B, FPN_POST_NMS_TOP_N, 5), np.float32)
    for b in range(B):
        order = np.argsort(-scores[b], kind="stable")[:FPN_POST_NMS_TOP_N]
        out[b, :, :4] = boxes[b][order]
        out[b, :, 4] = scores[b][order]
    return out
